# revision 1
# baseline (speedup 1.0000x reference)
"""TransE (KGE head-batch) scoring kernel for Trainium2, 8 NeuronCores.

score[b, n] = gamma - sum_d |E[head_part[b,n], d] + (R[tail_part[b,1], d]
                                                     - E[tail_part[b,2], d])|

Strategy: data-parallel over the batch dim (32 batches per core), entity
table replicated in each core's DRAM as bf16 (halves gather traffic; the
L1-sum tolerates bf16 rounding easily).  Per (batch, 128-neg tile) one
indirect SWDGE DMA gathers 128 embedding rows into SBUF; DVE adds the
per-batch (r - t) vector (broadcast via a K=1 PE matmul into PSUM); ACT
computes |.| with a fused free-dim accumulate; PE transposes the
[128 negs, 256 (b,tile)] score matrix back to output layout.
"""

import numpy as np
import ml_dtypes
from contextlib import ExitStack

import concourse.bass as bass
from concourse import mybir
from concourse.bass_utils import run_bass_kernel_spmd
from concourse.masks import make_identity

NCORES = 8
BATCH = 256
NEG = 1024
DIM = 512
NENTITY = 200000
NRELATION = 500

B_LOC = BATCH // NCORES          # 32 batches per core
NT = NEG // 128                  # 8 neg-tiles per batch
C = B_LOC * NT                   # 256 gather tiles (columns) per core
NB = 8                           # h/work ring depth

_DT_BF16 = mybir.dt.bfloat16
_DT_F32 = mybir.dt.float32
_DT_I32 = mybir.dt.int32


def build_nc() -> bass.Bass:
    nc = bass.Bass()
    etab = nc.dram_tensor("etab", [NENTITY, DIM], _DT_BF16, kind="ExternalInput")
    rtab = nc.dram_tensor("rtab", [NRELATION, DIM], _DT_F32, kind="ExternalInput")
    idx_t = nc.dram_tensor("idx_t", [128, C], _DT_I32, kind="ExternalInput")
    t1 = nc.dram_tensor("t1", [B_LOC, 1], _DT_I32, kind="ExternalInput")
    t2 = nc.dram_tensor("t2", [B_LOC, 1], _DT_I32, kind="ExternalInput")
    gam = nc.dram_tensor("gam", [128, 1], _DT_F32, kind="ExternalInput")
    scores_out = nc.dram_tensor("scores", [C, 128], _DT_F32, kind="ExternalOutput")
    rt_dram = nc.dram_tensor("rt_dram", [B_LOC, DIM], _DT_F32)

    with ExitStack() as st:
        en = st.enter_context
        # SBUF
        idx_sb = en(nc.sbuf_tensor("idx_sb", [128, C], _DT_I32))
        t1_sb = en(nc.sbuf_tensor("t1_sb", [B_LOC, 1], _DT_I32))
        t2_sb = en(nc.sbuf_tensor("t2_sb", [B_LOC, 1], _DT_I32))
        gam_sb = en(nc.sbuf_tensor("gam_sb", [128, 1], _DT_F32))
        r_rows = en(nc.sbuf_tensor("r_rows", [B_LOC, DIM], _DT_F32))
        t_rows = en(nc.sbuf_tensor("t_rows", [B_LOC, DIM], _DT_BF16))
        t_f32 = en(nc.sbuf_tensor("t_f32", [B_LOC, DIM], _DT_F32))
        rt_all = en(nc.sbuf_tensor("rt_all", [B_LOC, DIM], _DT_F32))
        ident = en(nc.sbuf_tensor("ident", [128, 128], _DT_F32))
        h_sb = en(nc.sbuf_tensor("h_sb", [128, NB, DIM], _DT_BF16))
        work = en(nc.sbuf_tensor("work", [128, NB, DIM], _DT_BF16))
        dummy = en(nc.sbuf_tensor("absdump", [128, DIM], _DT_BF16))
        rtb = en(nc.sbuf_tensor("rtb", [128, 2, DIM], _DT_BF16))
        sums = en(nc.sbuf_tensor("sums", [128, C], _DT_F32))
        sc_sb = en(nc.sbuf_tensor("sc_sb", [128, C], _DT_F32))
        out_sb = en(nc.sbuf_tensor("out_sb", [128, 2, 128], _DT_F32))
        # PSUM
        psum_t = [en(nc.psum_tensor(f"psum_t{i}", [128, 128], _DT_F32)) for i in range(2)]
        # Semaphores
        setup_sem = en(nc.semaphore("setup"))
        ld_sem = en(nc.semaphore("ld"))
        pre_sem = en(nc.semaphore("pre"))
        gsem = [en(nc.semaphore(f"g{i}")) for i in range(NB)]
        rts_sem = en(nc.semaphore("rts"))
        rtd_sem = en(nc.semaphore("rtd"))
        bsem = [en(nc.semaphore(f"b{i}")) for i in range(2)]
        add_sem = en(nc.semaphore("addd"))
        act_sem = en(nc.semaphore("act"))
        sc_sem = en(nc.semaphore("sc"))
        tr_sem = en(nc.semaphore("tr"))
        cp_sem = en(nc.semaphore("cp"))
        out_sem = en(nc.semaphore("outd"))

        # ---- setup constants (gpsimd) ----
        make_identity(nc, ident[:, :])  # memset + affine_select on gpsimd
        nc.gpsimd.tensor_copy(out=ident[:, 0:1], in_=ident[:, 0:1]).then_inc(
            setup_sem, 1
        )

        # ---- initial loads (gpsimd SWDGE) ----
        nc.gpsimd.dma_start(idx_sb[:, :], idx_t[:, :]).then_inc(ld_sem, 16)
        nc.gpsimd.dma_start(t1_sb[:, :], t1[:, :]).then_inc(ld_sem, 16)
        nc.gpsimd.dma_start(t2_sb[:, :], t2[:, :]).then_inc(ld_sem, 16)
        nc.gpsimd.dma_start(gam_sb[:, :], gam[:, :]).then_inc(ld_sem, 16)
        nc.gpsimd.wait_ge(ld_sem, 64)
        nc.gpsimd.indirect_dma_start(
            out=r_rows[:, :], out_offset=None, in_=rtab[:],
            in_offset=bass.IndirectOffsetOnAxis(ap=t1_sb[:, 0:1], axis=0),
        ).then_inc(pre_sem, 16)
        nc.gpsimd.indirect_dma_start(
            out=t_rows[:, :], out_offset=None, in_=etab[:],
            in_offset=bass.IndirectOffsetOnAxis(ap=t2_sb[:, 0:1], axis=0),
        ).then_inc(pre_sem, 16)

        # ---- rt = r - t on DVE ----
        nc.vector.wait_ge(pre_sem, 32)
        nc.vector.tensor_copy(out=t_f32[:, :], in_=t_rows[:, :])
        nc.vector.tensor_tensor(
            out=rt_all[:, :], in0=r_rows[:, :], in1=t_f32[:, :],
            op=mybir.AluOpType.subtract,
        ).then_inc(rts_sem, 1)

        # ---- stage rt to DRAM, then per-batch broadcast + gathers (gpsimd) ----
        nc.gpsimd.wait_ge(rts_sem, 1)
        nc.gpsimd.dma_start(rt_dram[:, :], rt_all[:, :]).then_inc(rtd_sem, 16)
        nc.gpsimd.wait_ge(rtd_sem, 16)

        def bcast_rt(b):
            # rtb[:, b%2, :] <- rt_dram[b, :] replicated to 128 partitions,
            # cast f32 -> bf16 in the SWDGE.
            src = bass.AP(rt_dram, b * DIM, [[0, 128], [1, DIM]])
            nc.gpsimd.dma_start(rtb[:, b % 2, :], src).then_inc(bsem[b % 2], 16)

        for b in range(B_LOC):
            if b >= 2:
                nc.gpsimd.wait_ge(add_sem, NT * (b - 1))
            bcast_rt(b)
            for t in range(NT):
                c = b * NT + t
                if c >= NB:
                    nc.gpsimd.wait_ge(add_sem, c - NB + 1)
                nc.gpsimd.indirect_dma_start(
                    out=h_sb[:, c % NB, :], out_offset=None, in_=etab[:],
                    in_offset=bass.IndirectOffsetOnAxis(
                        ap=idx_sb[:, c : c + 1], axis=0
                    ),
                ).then_inc(gsem[c % NB], 16)

        # ---- DVE: adds ----
        for b in range(B_LOC):
            nc.vector.wait_ge(bsem[b % 2], 16 * (b // 2 + 1))
            for t in range(NT):
                c = b * NT + t
                nc.vector.wait_ge(gsem[c % NB], 16 * (c // NB + 1))
                if c >= NB:
                    nc.vector.wait_ge(act_sem, c - NB + 1)
                nc.vector.tensor_tensor(
                    out=work[:, c % NB, :],
                    in0=h_sb[:, c % NB, :],
                    in1=rtb[:, b % 2, :],
                    op=mybir.AluOpType.add,
                ).then_inc(add_sem, 1)

        # ---- ACT: |.| with accumulate ----
        for c in range(C):
            nc.scalar.wait_ge(add_sem, c + 1)
            nc.scalar.activation(
                out=dummy[:, :],
                in_=work[:, c % NB, :],
                func=mybir.ActivationFunctionType.Abs,
                accum_out=sums[:, c : c + 1],
            ).then_inc(act_sem, 1)

        # ---- scores = gamma - sums (DVE) ----
        nc.vector.wait_ge(act_sem, C)
        nc.vector.tensor_scalar(
            out=sc_sb[:, :], in0=sums[:, :],
            scalar1=-1.0, scalar2=gam_sb[:, 0:1],
            op0=mybir.AluOpType.mult, op1=mybir.AluOpType.add,
        ).then_inc(sc_sem, 1)

        # ---- transpose + writeback ----
        nc.tensor.wait_ge(setup_sem, 1)
        nc.tensor.wait_ge(sc_sem, 1)
        for j in range(2):
            nc.tensor.transpose(
                out=psum_t[j][:, :],
                in_=sc_sb[:, j * 128 : (j + 1) * 128],
                identity=ident[:, :],
            ).then_inc(tr_sem, 1)
        for j in range(2):
            nc.vector.wait_ge(tr_sem, j + 1)
            nc.vector.tensor_copy(
                out=out_sb[:, j, :], in_=psum_t[j][:, :]
            ).then_inc(cp_sem, 1)
        for j in range(2):
            nc.gpsimd.wait_ge(cp_sem, j + 1)
            nc.gpsimd.dma_start(
                scores_out[j * 128 : (j + 1) * 128, :], out_sb[:, j, :]
            ).then_inc(out_sem, 16)
        nc.gpsimd.wait_ge(out_sem, 32)
    return nc


_NC = None


def _get_nc():
    global _NC
    if _NC is None:
        _NC = build_nc()
    return _NC


def _prep_inputs(tail_part, head_part, entity_embedding, relation_embedding, gamma):
    tail_part = np.asarray(tail_part)
    head_part = np.asarray(head_part)
    entity_embedding = np.asarray(entity_embedding)
    relation_embedding = np.asarray(relation_embedding)
    gamma = np.asarray(gamma)
    etab = np.ascontiguousarray(entity_embedding.astype(ml_dtypes.bfloat16))
    rtab = np.ascontiguousarray(relation_embedding.astype(np.float32))
    gam = np.full((128, 1), np.float32(gamma.reshape(-1)[0]), dtype=np.float32)
    head_i32 = head_part.astype(np.int32)         # [256, 1024]
    t1_all = tail_part[:, 1].astype(np.int32)
    t2_all = tail_part[:, 2].astype(np.int32)
    in_maps = []
    for k in range(NCORES):
        hb = head_i32[k * B_LOC : (k + 1) * B_LOC]          # [32, 1024]
        # idx_t[p, c] with c = b*NT + t  ->  hb[b, t*128 + p]
        idx_t = np.ascontiguousarray(
            hb.reshape(B_LOC, NT, 128).reshape(C, 128).T
        )  # [128, C]
        in_maps.append(
            {
                "etab": etab,
                "rtab": rtab,
                "idx_t": idx_t,
                "t1": np.ascontiguousarray(
                    t1_all[k * B_LOC : (k + 1) * B_LOC].reshape(B_LOC, 1)
                ),
                "t2": np.ascontiguousarray(
                    t2_all[k * B_LOC : (k + 1) * B_LOC].reshape(B_LOC, 1)
                ),
                "gam": gam,
            }
        )
    return in_maps


def _assemble(results):
    out = np.empty((BATCH, NEG), dtype=np.float32)
    for k in range(NCORES):
        sc = results[k]["scores"]  # [C, 128] rows c = b*NT + t
        out[k * B_LOC : (k + 1) * B_LOC] = sc.reshape(B_LOC, NT * 128)
    return out


def kernel(tail_part, head_part, entity_embedding, relation_embedding, gamma,
           **run_kwargs):
    nc = _get_nc()
    in_maps = _prep_inputs(
        tail_part, head_part, entity_embedding, relation_embedding, gamma
    )
    res = run_bass_kernel_spmd(
        nc, in_maps, core_ids=list(range(NCORES)), **run_kwargs
    )
    out = _assemble(res.results)
    if run_kwargs:
        return out, res
    return out

